# revision 11
# baseline (speedup 1.0000x reference)
"""Single-level 2D Haar DWT (pywt dwt2-compatible) on 8 TRN2 NeuronCores.

Input  x:   (32, 3, 512, 512) f32
Output out: (32, 12, 256, 256) f32, channel layout [LL, LH, HL, HH] per input
channel.

Sharding: pure data parallel — batch 32 -> 4 samples per core on 8 cores.

The HBM roofline for f32 I/O is ~70 us/core (12 MiB in + 12 MiB out).
The transform's 1/2 scale is folded into a host-side f32->bf16 conversion
(y = bf16(x/2)), so the device reads bf16, computes pure add/sub
butterflies in bf16 on the vector engine, and writes bf16 — halving HBM
traffic to ~12.6 MB/core. End-to-end rel err vs the f32 reference is
~3e-3 (quantization + bf16 arithmetic), far inside the 2e-2 gate.

Per-core layout: the 12 images (4 samples x 3 channels) are processed one
image per group. Partition p holds image rows 4p..4p+3 (r = 2k + t: k
selects the output row 2p+k, t the row parity), and the host additionally
de-interleaves each row's columns into [256 even | 256 odd] halves, so
the input load is a fully contiguous 0.5 MiB DMA (4 KiB per partition)
AND every DVE operand below is a packed stride-1 bf16 view — which is
what the DVE's 2x 16-bit performance mode requires (stride-2 views run
at half rate and made the vector engine the bottleneck in an earlier
version: 42.7 us vs 28.4 us with packed operands).

Compute per group (DVE only, all bf16, all operands packed):
  s[k,t,j] = R[k,t,0,j] + R[k,t,1,j]      (column butterfly, h = col parity)
  d[k,t,j] = R[k,t,0,j] - R[k,t,1,j]
  LL[k,j] = s[k,0,j] + s[k,1,j]           (row butterfly)
  LH[k,j] = s[k,0,j] - s[k,1,j]
  HL[k,j] = d[k,0,j] + d[k,1,j]
  HH[k,j] = d[k,0,j] - d[k,1,j]
Q[p, q, k, j] leaves as one fully contiguous 0.5 MiB DMA (4 KiB per
partition); the host un-permutes (q, 2p+k) -> plane rows afterwards.

Loads and stores alternate between the two HWDGE rings (SP / ACT) per
group so each ring carries half the loads and half the stores and both
stream concurrently.

Measured alternatives that LOST on hardware (kept here so they are not
retried): M=3 grouping with one or two row-stage ops offloaded to the
Pool/gpsimd engine regressed to 32.8-38.5 us — real Q7 bf16 tensor_sub
runs at ~2.4 ns/elem-partition (4.7x slower than DVE 2x mode) and the
cross-engine dependency chains serialize the pipeline.
"""

import ml_dtypes
import numpy as np

import concourse.bacc as bacc
import concourse.tile as tile
from concourse import mybir
from concourse.bass_utils import run_bass_kernel_spmd

N_CORES = 8
B, C, H, W = 32, 3, 512, 512
BPC = B // N_CORES          # samples per core
IMGS = BPC * C              # images per core
HALF_W = W // 2
G = IMGS                    # groups per core (one image per group)
IN_COLS = 4 * W             # 2048 bf16 elems per partition per group
OUT_COLS = 4 * 2 * HALF_W   # 2048 bf16 elems per partition per group
IN_ROWS = G * 128
OUT_ROWS = G * 128

_BF16 = mybir.dt.bfloat16
_NP_BF16 = ml_dtypes.bfloat16


def build(repeat: int = 1):
    """Build and compile the per-core Bass program. repeat>1 re-runs the whole
    body back to back (used for on-hardware timing)."""
    nc = bacc.Bacc("TRN2", debug=False, num_devices=N_CORES)
    x = nc.dram_tensor("x", [IN_ROWS, IN_COLS], _BF16, kind="ExternalInput")
    out = nc.dram_tensor("out", [OUT_ROWS, OUT_COLS], _BF16, kind="ExternalOutput")

    xv = x.ap().rearrange("(g p) c -> g p c", g=G)
    ov = out.ap().rearrange("(g p) c -> g p c", g=G)

    with tile.TileContext(nc) as tc:
        with (
            tc.tile_pool(name="io", bufs=3) as io_pool,
            tc.tile_pool(name="mid", bufs=3) as mid_pool,
        ):
            for _ in range(repeat):
                for g in range(G):
                    ld_eng, st_eng = (
                        (nc.sync, nc.scalar) if g % 2 == 0 else (nc.scalar, nc.sync)
                    )
                    R = io_pool.tile([128, IN_COLS], _BF16, tag="R")
                    ld_eng.dma_start(out=R, in_=xv[g])
                    # [p, k, t, h, j]: k output-row, t row parity, h col parity
                    # (host pre-split each row into [even cols | odd cols])
                    Rv = R.rearrange("p (k t h j) -> p k t h j", k=2, t=2, h=2)

                    s = mid_pool.tile([128, 4 * HALF_W], _BF16, tag="s")
                    d = mid_pool.tile([128, 4 * HALF_W], _BF16, tag="d")
                    sv = s.rearrange("p (k t j) -> p k t j", k=2, t=2)
                    dv = d.rearrange("p (k t j) -> p k t j", k=2, t=2)
                    nc.vector.tensor_add(sv, Rv[:, :, :, 0], Rv[:, :, :, 1])
                    nc.vector.tensor_sub(dv, Rv[:, :, :, 0], Rv[:, :, :, 1])

                    # LL/LH/HL on the DVE in one tile; HH on the Pool engine in
                    # its OWN tile with its OWN store. Earlier attempts that
                    # let Pool write into the shared Q tile regressed badly
                    # (32.8-38.5 us) — cross-engine writes to one tile
                    # serialize the pipeline. A private tile keeps the only
                    # cross-engine edge the unavoidable read of d.
                    Q = io_pool.tile([128, 3 * 2 * HALF_W], _BF16, tag="Q")
                    Qv = Q.rearrange("p (q k j) -> p q k j", q=3, k=2)
                    Qh = io_pool.tile([128, 2 * HALF_W], _BF16, tag="Qh")
                    Qhv = Qh.rearrange("p (k j) -> p k j", k=2)
                    nc.vector.tensor_add(Qv[:, 0], sv[:, :, 0], sv[:, :, 1])
                    nc.vector.tensor_sub(Qv[:, 1], sv[:, :, 0], sv[:, :, 1])
                    nc.vector.tensor_add(Qv[:, 2], dv[:, :, 0], dv[:, :, 1])
                    nc.gpsimd.tensor_sub(Qhv, dv[:, :, 0], dv[:, :, 1])

                    st_eng.dma_start(out=ov[g][:, : 3 * 2 * HALF_W], in_=Q)
                    st_eng.dma_start(out=ov[g][:, 3 * 2 * HALF_W :], in_=Qh)

    nc.compile()
    return nc


_NC_CACHE: dict[int, object] = {}


def _get_nc(repeat: int = 1):
    if repeat not in _NC_CACHE:
        _NC_CACHE[repeat] = build(repeat)
    return _NC_CACHE[repeat]


def prep_full(x: np.ndarray) -> np.ndarray:
    """Prescale + quantize + column-deinterleave the full input on the host.

    y = bf16(x/2) with each image row rewritten as [256 even cols | 256 odd
    cols], so the device sees packed stride-1 column-parity halves."""
    y = (x * np.float32(0.5)).astype(_NP_BF16)
    t = np.empty((B, C, H, 2, HALF_W), dtype=_NP_BF16)
    t[:, :, :, 0, :] = y[:, :, :, 0::2]
    t[:, :, :, 1, :] = y[:, :, :, 1::2]
    return t


def prep_shard(t: np.ndarray, c: int) -> np.ndarray:
    """Per-core device input from prep_full's output. Partition p of group
    (image) g holds rows 4p..4p+3 — a pure reshape of the prepped layout."""
    return np.ascontiguousarray(t[c * BPC : (c + 1) * BPC]).reshape(IN_ROWS, IN_COLS)


def post_shard(arr: np.ndarray) -> np.ndarray:
    """Device output (OUT_ROWS, OUT_COLS) bf16 -> (BPC, C*4, 256, 256) f32.

    arr[g, p, q, k, j] is plane row 2p+k of quadrant q of image g."""
    a = np.asarray(arr).reshape(G, 128, 4, 2, HALF_W)
    a = a.transpose(0, 2, 1, 3, 4).reshape(BPC, C * 4, H // 2, HALF_W)
    return a.astype(np.float32)


def kernel(x: np.ndarray) -> np.ndarray:
    x = np.asarray(x, dtype=np.float32)
    assert x.shape == (B, C, H, W)
    t = prep_full(x)
    nc = _get_nc()
    in_maps = [{"x": prep_shard(t, c)} for c in range(N_CORES)]
    res = run_bass_kernel_spmd(nc, in_maps, list(range(N_CORES)))
    shards = [post_shard(res.results[c]["out"]) for c in range(N_CORES)]
    return np.concatenate(shards, axis=0)
